# revision 4
# baseline (speedup 1.0000x reference)
"""Multi-head attention (B=2, S=2048, D=1024, H=16) Bass kernel for 8 Trainium2 cores.

Sharding: core c handles batch b = c//4 and head group g = c%4 (4 heads,
d-slice [256g, 256g+256)).  Data-parallel over batch, tensor-parallel over
heads; w_o is row-parallel with an on-device ReduceScatter over each batch's
4 cores, followed by residual + LayerNorm on each core's 512 rows.

Per core (all fp32 except the additive mask, bf16):
  phase 0: x^T via PE transposes; Q^T,K^T (per-head-transposed, Q pre-scaled
           by 1/8 with bias folded) and V (natural) projections.
  phase A: scores S[q,k] = Q^T.T @ K^T (+ additive -1e9 mask via a bf16
           identity-matmul into the same PSUM group), exp on ACT with fused
           row-sum accumulation, reciprocal, normalize, DMA out -> attention.
  phase B: scores recomputed transposed ST[k,q] (+ mask^T streamed from DRAM),
           exp, PV matmul off the un-normalized exp; normalization applied to
           ctx^T via a per-partition-built, PE-transposed reciprocal outer
           product.  Head pairs sit at partitions 0-63/64-127 so their score
           matmuls occupy different PE row groups (issued adjacently).
  phase C: out = ctx^T.T @ wo^T (partial), ReduceScatter(sum) over the
           batch's 4 cores, + bias + residual, LayerNorm, DMA out.

The attention matrix is written in its natural [q,k] layout (it is a kernel
output); recomputing the transposed scores on the PE is far cheaper than
transposing 64 MiB/core of probabilities.
"""

import numpy as np
import ml_dtypes

import concourse.bass as bass
import concourse.mybir as mybir
import concourse.tile as tile
from concourse import bacc

F32 = mybir.dt.float32
BF16 = mybir.dt.bfloat16
AF = mybir.ActivationFunctionType
ALU = mybir.AluOpType

B, S, D = 2, 2048, 1024
NH = 16            # total heads
HPC = 4            # heads per core
DS = HPC * 64      # 256, d-slice width per core
N_CORES = 8
ROWS = S // 4      # 512 output rows per core
BIG = 1e9
EPS = 1e-5

QT_TILES = DS // 128   # 2
ECH = D // 128         # 8  e-chunks
ST_T = S // 128        # 16 s/q/k tiles
NC4 = S // 512         # 4  512-wide chunks

_CACHE = {}


def build_nc():
    nc = bacc.Bacc("TRN2", target_bir_lowering=False, debug=False,
                   enable_asserts=False, num_devices=N_CORES)

    # ---- I/O ----
    xq = nc.declare_dram_parameter("xq", [S, D], F32, isOutput=False)
    xk = nc.declare_dram_parameter("xk", [S, D], F32, isOutput=False)
    xv = nc.declare_dram_parameter("xv", [S, D], F32, isOutput=False)
    a_m = nc.declare_dram_parameter("a_m", [S, S], BF16, isOutput=False)    # (m-1)*1e9, [q,k]
    a_mt = nc.declare_dram_parameter("a_mt", [S, S], BF16, isOutput=False)  # transposed, [k,q]
    wqT = nc.declare_dram_parameter("wqT", [D, DS], F32, isOutput=False)
    wkT = nc.declare_dram_parameter("wkT", [D, DS], F32, isOutput=False)
    wvT = nc.declare_dram_parameter("wvT", [D, DS], F32, isOutput=False)
    woT = nc.declare_dram_parameter("woT", [DS, D], F32, isOutput=False)
    bq8 = nc.declare_dram_parameter("bq8", [DS], F32, isOutput=False)  # bq/8
    bk4 = nc.declare_dram_parameter("bk4", [DS], F32, isOutput=False)
    bv4 = nc.declare_dram_parameter("bv4", [DS], F32, isOutput=False)
    bo = nc.declare_dram_parameter("bo", [D], F32, isOutput=False)
    gamma = nc.declare_dram_parameter("gamma", [D], F32, isOutput=False)
    beta = nc.declare_dram_parameter("beta", [D], F32, isOutput=False)
    resid = nc.declare_dram_parameter("resid", [ROWS, D], F32, isOutput=False)

    attn = nc.declare_dram_parameter("attn", [HPC, S, S], F32, isOutput=True)
    out_rows = nc.declare_dram_parameter("out_rows", [ROWS, D], F32, isOutput=True)

    with tile.TileContext(nc) as tc:
        with (
            tc.tile_pool(name="cn", bufs=1) as cn,
            tc.tile_pool(name="res", bufs=1) as res,
            tc.tile_pool(name="ps", bufs=4, space="PSUM") as ps,
            tc.tile_pool(name="psc", bufs=4, space="PSUM") as psc,
            tc.tile_pool(name="dr", bufs=1, space="DRAM") as dr,
        ):
            # ---------------- constants ----------------
            ident_f = cn.tile([128, 128], F32, tag="idf")
            ident_b = cn.tile([128, 128], BF16, tag="idb")
            from concourse.masks import make_identity
            make_identity(nc, ident_f)
            make_identity(nc, ident_b)
            ones = cn.tile([128, 128], F32, tag="ones")
            nc.vector.memset(ones, 1.0)
            eps_t = cn.tile([128, 1], F32, tag="eps")
            nc.vector.memset(eps_t, EPS)

            woT_t = cn.tile([128, QT_TILES, D], F32, tag="woT")
            nc.sync.dma_start(out=woT_t, in_=woT.ap().rearrange("(c p) d -> p c d", p=128))
            bq8_t = cn.tile([128, QT_TILES], F32, tag="bq8")
            nc.sync.dma_start(out=bq8_t, in_=bq8.ap().rearrange("(c p) -> p c", p=128))
            bk4_t = cn.tile([128, QT_TILES], F32, tag="bk4")
            nc.sync.dma_start(out=bk4_t, in_=bk4.ap().rearrange("(c p) -> p c", p=128))

            def _bcast(handle, n):
                a = handle.ap()
                return bass.AP(tensor=a.tensor, offset=a.offset,
                               ap=[[0, 128], [1, n]])

            bv4_bc = cn.tile([128, DS], F32, tag="bv4")
            nc.gpsimd.dma_start(out=bv4_bc, in_=_bcast(bv4, DS))
            bo_bc = cn.tile([128, D], F32, tag="bo")
            nc.gpsimd.dma_start(out=bo_bc, in_=_bcast(bo, D))
            ga_bc = cn.tile([128, D], F32, tag="ga")
            nc.gpsimd.dma_start(out=ga_bc, in_=_bcast(gamma, D))
            be_bc = cn.tile([128, D], F32, tag="be")
            nc.gpsimd.dma_start(out=be_bc, in_=_bcast(beta, D))

            # ---------------- long-lived activations ----------------
            QT = res.tile([128, QT_TILES, S], F32, tag="QT")   # [d%128, d//128, q]
            KT = res.tile([128, QT_TILES, S], F32, tag="KT")
            V = res.tile([128, ST_T, DS], F32, tag="V")        # [k%128, k//128, d]
            ctxT = res.tile([128, QT_TILES, S], F32, tag="ctxT")
            rstore = res.tile([128, HPC, ST_T], F32, tag="rstore")  # recip rowsums

            # ---------------- phase 0: x^T + projections ----------------
            with (
                tc.tile_pool(name="wt", bufs=1) as wt,
                tc.tile_pool(name="xw", bufs=2) as xw,
                tc.tile_pool(name="xt", bufs=2) as xt,
            ):
                wqT_t = wt.tile([128, ECH, DS], F32, tag="wq")
                wkT_t = wt.tile([128, ECH, DS], F32, tag="wk")
                wvT_t = wt.tile([128, ECH, DS], F32, tag="wv")
                nc.sync.dma_start(out=wqT_t, in_=wqT.ap().rearrange("(c p) d -> p c d", p=128))
                nc.sync.dma_start(out=wkT_t, in_=wkT.ap().rearrange("(c p) d -> p c d", p=128))
                nc.sync.dma_start(out=wvT_t, in_=wvT.ap().rearrange("(c p) d -> p c d", p=128))

                for tname, xh, outQ in (("q", xq, 0), ("k", xk, 1), ("v", xv, 2)):
                    for n4 in range(NC4):
                        xblk = xw.tile([128, 4, D], F32, tag="xblk",
                                       name=f"xblk_{tname}_{n4}")
                        for j in range(4):
                            st = 4 * n4 + j
                            nc.sync.dma_start(out=xblk[:, j, :],
                                              in_=xh[st * 128:(st + 1) * 128, :])
                        xTc = xt.tile([128, ECH, 512], F32, tag="xTc",
                                      name=f"xTc_{tname}_{n4}")
                        for ec in range(ECH):
                            pst = ps.tile([128, 512], F32, tag="ps",
                                          name=f"pt_{tname}_{n4}_{ec}")
                            for j in range(4):
                                nc.tensor.transpose(
                                    pst[:, j * 128:(j + 1) * 128],
                                    xblk[:, j, ec * 128:(ec + 1) * 128],
                                    ident_f)
                            nc.scalar.copy(xTc[:, ec, :], pst)
                        if outQ == 0:  # Q^T scaled by 1/8, bias bq/8
                            for m2 in range(QT_TILES):
                                psq = ps.tile([128, 512], F32, tag="ps",
                                              name=f"psq_{n4}_{m2}")
                                for ec in range(ECH):
                                    nc.tensor.matmul(
                                        psq, wqT_t[:, ec, m2 * 128:(m2 + 1) * 128],
                                        xTc[:, ec, :],
                                        start=(ec == 0), stop=(ec == ECH - 1))
                                nc.scalar.activation(
                                    QT[:, m2, n4 * 512:(n4 + 1) * 512], psq,
                                    AF.Identity, bias=bq8_t[:, m2:m2 + 1], scale=0.125)
                        elif outQ == 1:  # K^T
                            for m2 in range(QT_TILES):
                                psk = ps.tile([128, 512], F32, tag="ps",
                                              name=f"psk_{n4}_{m2}")
                                for ec in range(ECH):
                                    nc.tensor.matmul(
                                        psk, wkT_t[:, ec, m2 * 128:(m2 + 1) * 128],
                                        xTc[:, ec, :],
                                        start=(ec == 0), stop=(ec == ECH - 1))
                                nc.scalar.activation(
                                    KT[:, m2, n4 * 512:(n4 + 1) * 512], psk,
                                    AF.Identity, bias=bk4_t[:, m2:m2 + 1], scale=1.0)
                        else:  # V natural [s, d]
                            for j in range(4):
                                st = 4 * n4 + j
                                psv = ps.tile([128, 512], F32, tag="ps",
                                              name=f"psv_{n4}_{j}")
                                for ec in range(ECH):
                                    nc.tensor.matmul(
                                        psv[:, 0:DS],
                                        xTc[:, ec, j * 128:(j + 1) * 128],
                                        wvT_t[:, ec, :],
                                        start=(ec == 0), stop=(ec == ECH - 1))
                                nc.vector.scalar_tensor_tensor(
                                    V[:, st, :], psv[:, 0:DS], 1.0, bv4_bc,
                                    ALU.mult, ALU.add)

            with (
                tc.tile_pool(name="wka", bufs=2) as wka,
                tc.tile_pool(name="wkp", bufs=4) as wkp,
                tc.tile_pool(name="wac", bufs=8) as wac,
                tc.tile_pool(name="atk", bufs=3) as atk,
                tc.tile_pool(name="wpt", bufs=4) as wpt,
                tc.tile_pool(name="wr", bufs=2) as wr,
                tc.tile_pool(name="wo_", bufs=3) as wo_,
            ):
                # ------------ phase A: natural-layout scores -> attention ----
                for qt in range(ST_T):
                    a_qt = wka.tile([128, S], BF16, tag="aqt", name=f"aqt_{qt}")
                    nc.sync.dma_start(out=a_qt, in_=a_m[qt * 128:(qt + 1) * 128, :])
                    for pr in range(2):  # head pair: partitions 0-63 / 64-127
                        prow = [wkp.tile([128, S], F32, tag="prow",
                                         name=f"prow_{qt}_{pr}_{i}") for i in range(2)]
                        accs = [wac.tile([128, NC4 + 1], F32, tag="acc",
                                         name=f"acc_{qt}_{pr}_{i}") for i in range(2)]
                        for n4 in range(NC4):
                            pss = [ps.tile([128, 512], F32, tag="ps",
                                           name=f"pss_{qt}_{pr}_{n4}_{i}") for i in range(2)]
                            for h2 in range(2):  # adjacent MMs, distinct row groups
                                ho = 64 * h2
                                nc.tensor.matmul(
                                    pss[h2],
                                    QT[ho:ho + 64, pr, qt * 128:(qt + 1) * 128],
                                    KT[ho:ho + 64, pr, n4 * 512:(n4 + 1) * 512],
                                    start=True, stop=False)
                            for h2 in range(2):
                                nc.tensor.matmul(
                                    pss[h2], ident_b, a_qt[:, n4 * 512:(n4 + 1) * 512],
                                    start=False, stop=True)
                            for h2 in range(2):
                                nc.scalar.activation(
                                    prow[h2][:, n4 * 512:(n4 + 1) * 512], pss[h2],
                                    AF.Exp, scale=1.0, accum_out=accs[h2][:, n4:n4 + 1])
                        for h2 in range(2):
                            h = 2 * pr + h2
                            nc.vector.tensor_reduce(
                                accs[h2][:, NC4:NC4 + 1], accs[h2][:, 0:NC4],
                                axis=mybir.AxisListType.X, op=ALU.add)
                            nc.vector.reciprocal(rstore[:, h, qt:qt + 1],
                                                 accs[h2][:, NC4:NC4 + 1])
                            nc.vector.tensor_scalar(prow[h2], prow[h2],
                                                    rstore[:, h, qt:qt + 1], None,
                                                    ALU.mult)
                            nc.sync.dma_start(
                                out=attn[h, qt * 128:(qt + 1) * 128, :],
                                in_=prow[h2])

                # ------------ phase B: transposed scores -> ctx^T ------------
                for qh in range(2):  # q halves: qc in {2qh, 2qh+1}
                    ctx_ps = [[psc.tile([128, 512], F32, tag="ctx",
                                        name=f"ctx_{qh}_{pr}_{i}") for i in range(2)]
                              for pr in range(2)]
                    for kt in range(ST_T):
                        at_kt = atk.tile([128, S], BF16, tag="atk",
                                         name=f"atk_{qh}_{kt}")
                        nc.sync.dma_start(out=at_kt,
                                          in_=a_mt[kt * 128:(kt + 1) * 128, :])
                        for pr in range(2):
                            for qc2 in range(2):
                                qc = 2 * qh + qc2
                                pst2 = [ps.tile([128, 512], F32, tag="ps",
                                                name=f"pst_{qh}_{kt}_{pr}_{qc2}_{i}")
                                        for i in range(2)]
                                for h2 in range(2):
                                    ho = 64 * h2
                                    nc.tensor.matmul(
                                        pst2[h2],
                                        KT[ho:ho + 64, pr, kt * 128:(kt + 1) * 128],
                                        QT[ho:ho + 64, pr, qc * 512:(qc + 1) * 512],
                                        start=True, stop=False)
                                for h2 in range(2):
                                    nc.tensor.matmul(
                                        pst2[h2], ident_b,
                                        at_kt[:, qc * 512:(qc + 1) * 512],
                                        start=False, stop=True)
                                pt2 = [wpt.tile([128, 512], F32, tag="pt",
                                                name=f"ptx_{qh}_{kt}_{pr}_{qc2}_{i}")
                                       for i in range(2)]
                                for h2 in range(2):
                                    nc.scalar.activation(pt2[h2], pst2[h2], AF.Exp,
                                                         scale=1.0)
                                for h2 in range(2):
                                    h = 2 * pr + h2
                                    nc.tensor.matmul(
                                        ctx_ps[pr][qc2][64 * h2:64 * h2 + 64, :],
                                        V[:, kt, h * 64:(h + 1) * 64], pt2[h2],
                                        start=(kt == 0), stop=(kt == ST_T - 1))
                    # normalize ctx^T: R[d,q] built as R^T per-partition, then
                    # PE-transposed.
                    for pr in range(2):
                        for qc2 in range(2):
                            qc = 2 * qh + qc2
                            psr = ps.tile([128, 512], F32, tag="ps",
                                          name=f"psr_{qh}_{pr}_{qc2}")
                            for j in range(4):
                                qt = 4 * qc + j
                                rT = wr.tile([128, 128], F32, tag="rT",
                                             name=f"rT_{qh}_{pr}_{qc2}_{j}")
                                nc.vector.tensor_scalar(
                                    rT[:, 0:64], ones[:, 0:64],
                                    rstore[:, 2 * pr, qt:qt + 1], None, ALU.mult)
                                nc.vector.tensor_scalar(
                                    rT[:, 64:128], ones[:, 64:128],
                                    rstore[:, 2 * pr + 1, qt:qt + 1], None, ALU.mult)
                                nc.tensor.transpose(
                                    psr[:, j * 128:(j + 1) * 128], rT, ident_f)
                            r_sb = wr.tile([128, 512], F32, tag="rsb",
                                           name=f"rsb_{qh}_{pr}_{qc2}")
                            nc.scalar.copy(r_sb, psr)
                            nc.vector.scalar_tensor_tensor(
                                ctxT[:, pr, qc * 512:(qc + 1) * 512],
                                ctx_ps[pr][qc2], 1.0, r_sb, ALU.mult, ALU.mult)

                # ------------ phase C: out proj + RS + LN --------------------
                partial_d = dr.tile([S, D], F32, tag="partial")
                rs_d = dr.tile([ROWS, D], F32, tag="rsout")
                for st in range(ST_T):
                    o_sb = wo_.tile([128, D], F32, tag="osb", name=f"osb_{st}")
                    for n2 in range(2):
                        pso = ps.tile([128, 512], F32, tag="ps",
                                      name=f"pso_{st}_{n2}")
                        for hp in range(QT_TILES):
                            nc.tensor.matmul(
                                pso, ctxT[:, hp, st * 128:(st + 1) * 128],
                                woT_t[:, hp, n2 * 512:(n2 + 1) * 512],
                                start=(hp == 0), stop=(hp == QT_TILES - 1))
                        nc.scalar.copy(o_sb[:, n2 * 512:(n2 + 1) * 512], pso)
                    nc.sync.dma_start(
                        out=partial_d[st * 128:(st + 1) * 128, :], in_=o_sb)
                nc.gpsimd.collective_compute(
                    "ReduceScatter", ALU.add,
                    replica_groups=[[0, 1, 2, 3], [4, 5, 6, 7]],
                    ins=[partial_d.opt()], outs=[rs_d.opt()])
                for rt in range(ROWS // 128):
                    x_sb = wo_.tile([128, D], F32, tag="xsb", name=f"xsb_{rt}")
                    r_sb2 = wo_.tile([128, D], F32, tag="resid", name=f"rsd_{rt}")
                    nc.sync.dma_start(out=x_sb, in_=rs_d[rt * 128:(rt + 1) * 128, :])
                    nc.sync.dma_start(out=r_sb2, in_=resid[rt * 128:(rt + 1) * 128, :])
                    nc.vector.tensor_add(x_sb, x_sb, r_sb2)
                    nc.vector.tensor_add(x_sb, x_sb, bo_bc)
                    stats = wac.tile([128, 2, 6], F32, tag="stats", name=f"st_{rt}")
                    mv = wac.tile([128, 2], F32, tag="mv", name=f"mv_{rt}")
                    for sg in range(2):
                        nc.vector.bn_stats(stats[:, sg, :],
                                           x_sb[:, sg * 512:(sg + 1) * 512])
                    nc.vector.bn_aggr(mv, stats)
                    sd = wac.tile([128, 2], F32, tag="sd", name=f"sd_{rt}")
                    nc.scalar.activation(sd[:, 0:1], mv[:, 1:2], AF.Sqrt,
                                         bias=eps_t, scale=1.0)
                    nc.vector.reciprocal(sd[:, 1:2], sd[:, 0:1])
                    nc.vector.tensor_scalar(x_sb, x_sb, mv[:, 0:1], sd[:, 1:2],
                                            ALU.subtract, ALU.mult)
                    nc.vector.scalar_tensor_tensor(x_sb, x_sb, 1.0, ga_bc,
                                                   ALU.mult, ALU.mult)
                    nc.vector.tensor_add(x_sb, x_sb, be_bc)
                    nc.sync.dma_start(
                        out=out_rows[rt * 128:(rt + 1) * 128, :], in_=x_sb)

    nc.compile()
    return nc


def make_in_maps(q, k, v, attn_mask, wq, bq, wk, bk, wv, bv, wo, bo, gamma, beta):
    bf = ml_dtypes.bfloat16
    a_full, at_full = [], []
    for b in range(B):
        m = attn_mask[b].astype(np.float32)
        a = ((m - 1.0) * BIG).astype(bf)
        a_full.append(a)
        at_full.append(np.ascontiguousarray(a.T))
    in_maps = []
    for c in range(N_CORES):
        b, g = c // 4, c % 4
        ds = slice(DS * g, DS * (g + 1))
        rows = slice(ROWS * g, ROWS * (g + 1))
        in_maps.append({
            "xq": np.ascontiguousarray(q[b]),
            "xk": np.ascontiguousarray(k[b]),
            "xv": np.ascontiguousarray(v[b]),
            "a_m": a_full[b],
            "a_mt": at_full[b],
            "wqT": np.ascontiguousarray(wq[ds].T),
            "wkT": np.ascontiguousarray(wk[ds].T),
            "wvT": np.ascontiguousarray(wv[ds].T),
            "woT": np.ascontiguousarray(wo[:, ds].T),
            "bq8": np.ascontiguousarray(bq[ds] / 8.0),
            "bk4": np.ascontiguousarray(bk[ds]),
            "bv4": np.ascontiguousarray(bv[ds]),
            "bo": np.ascontiguousarray(bo),
            "gamma": np.ascontiguousarray(gamma),
            "beta": np.ascontiguousarray(beta),
            "resid": np.ascontiguousarray(q[b, rows]),
        })
    return in_maps


def kernel(q, k, v, attn_mask, wq, bq, wk, bk, wv, bv, wo, bo, gamma, beta):
    from concourse.bass_utils import run_bass_kernel_spmd

    args = [np.asarray(x, dtype=np.float32) for x in
            (q, k, v)] + [np.asarray(attn_mask, dtype=np.int32)] + \
           [np.asarray(x, dtype=np.float32) for x in
            (wq, bq, wk, bk, wv, bv, wo, bo, gamma, beta)]
    q, k, v, attn_mask, wq, bq, wk, bk, wv, bv, wo, bo, gamma, beta = args

    if "nc" not in _CACHE:
        _CACHE["nc"] = build_nc()
    nc = _CACHE["nc"]

    in_maps = make_in_maps(q, k, v, attn_mask, wq, bq, wk, bk, wv, bv,
                           wo, bo, gamma, beta)
    res = run_bass_kernel_spmd(nc, in_maps, core_ids=list(range(N_CORES)))

    output = np.empty((B, S, D), np.float32)
    attention = np.empty((B, NH, S, S), np.float32)
    for c in range(N_CORES):
        b, g = c // 4, c % 4
        attention[b, HPC * g:HPC * (g + 1)] = res.results[c]["attn"]
        output[b, ROWS * g:ROWS * (g + 1)] = res.results[c]["out_rows"]
    return output, attention
